# revision 20
# baseline (speedup 1.0000x reference)
"""Trainium2 Bass kernel for nn_MultiHeadConvAttention.

Math collapse used here (verified against the reference):
  - The 5x5 attention conv is linear, so conv(qh_i + kh_j) = conv(qh_i) + conv(kh_j).
  - Softmax over keys j is invariant to the query-side term conv(qh_i) (constant in j)
    and to b_attn, so attn[b,n,i,j,h,w] = softmax_j(conv(kh)[b,n,j,h,w]) — independent
    of i and of q entirely. Likewise the weighted V-sum and the final fc are
    independent of the query index, so out[b,q,...] is constant over q.

Sharding: core = b*4 + n handles one (batch, head) pair. Device work per core:
  kh/vh projections with bias folded in as an extra contraction row (per-w-column
  matmuls into a (w_sub, c)-partition layout), leaky-relu on DVE, the 5x5 conv as a
  W-Toeplitz matmul over (w_in, c) + a small shifted-add over dh, exp + key-sum,
  a replication matmul, the attn*V contraction on DVE, and the final fc as a
  block-diagonal matmul. Unnormalized exp E and key-sums S are returned;
  normalization/broadcast of the attn output happens on the host.

SBUF free layout convention is (h, j): innermost j so key-reductions are contiguous.
"""

import sys

for _p in ("/opt/trn_rl_repo", "/opt/pypackages"):
    if _p not in sys.path:
        sys.path.insert(0, _p)

import numpy as np

B, L, CF, NH, DM, H, W = 2, 24, 32, 4, 32, 16, 16
HW = H * W
HJ = H * L  # 384, free size of (h, j)
N_CORES = 8

_RUNNER = None


def _build_nc():
    import concourse.bacc as bacc
    import concourse.mybir as mybir
    import concourse.tile as tile
    from contextlib import ExitStack

    f32 = mybir.dt.float32
    nc = bacc.Bacc("TRN2", target_bir_lowering=False, debug=False)

    kx = nc.dram_tensor("kx", [L, CF + 1, HW], f32, kind="ExternalInput")
    vx = nc.dram_tensor("vx", [L, CF + 1, HW], f32, kind="ExternalInput")
    wkT = nc.dram_tensor("wkT", [CF + 1, DM], f32, kind="ExternalInput")
    wvT = nc.dram_tensor("wvT", [CF + 1, DM], f32, kind="ExternalInput")
    cw = nc.dram_tensor("cw", [4, 128, 144], f32, kind="ExternalInput")
    rep = nc.dram_tensor("rep", [4, 16, 128], f32, kind="ExternalInput")
    fcw = nc.dram_tensor("fcw", [128, 128], f32, kind="ExternalInput")
    outp = nc.dram_tensor("outp", [W, DM, H], f32, kind="ExternalOutput")
    es = nc.dram_tensor("es", [W, HJ + H], f32, kind="ExternalOutput")

    Act = mybir.ActivationFunctionType
    Alu = mybir.AluOpType
    Ax = mybir.AxisListType

    with tile.TileContext(nc) as tc, ExitStack() as ctx:
        const = ctx.enter_context(tc.tile_pool(name="const", bufs=1))
        data = ctx.enter_context(tc.tile_pool(name="data", bufs=1))
        tmpp = ctx.enter_context(tc.tile_pool(name="tmpp", bufs=2))
        ps_proj = ctx.enter_context(tc.tile_pool(name="ps_proj", bufs=2, space="PSUM"))
        ps_p = ctx.enter_context(tc.tile_pool(name="ps_p", bufs=1, space="PSUM"))
        ps_er = ctx.enter_context(tc.tile_pool(name="ps_er", bufs=2, space="PSUM"))
        ps_o = ctx.enter_context(tc.tile_pool(name="ps_o", bufs=1, space="PSUM"))

        # ---- constant tables ----
        wkT_sb = const.tile([CF + 1, DM], f32, tag="wkT_sb")
        nc.sync.dma_start(wkT_sb[:], wkT.ap())
        wvT_sb = const.tile([CF + 1, DM], f32, tag="wvT_sb")
        nc.sync.dma_start(wvT_sb[:], wvT.ap())
        cw_sb = const.tile([128, 4 * 144], f32, tag="cw_sb")
        nc.sync.dma_start(
            cw_sb[:].rearrange("k (t m) -> k t m", t=4),
            cw.ap().rearrange("t k m -> k t m"),
        )
        rep_sb = const.tile([16, 4 * 128], f32, tag="rep_sb")
        nc.sync.dma_start(
            rep_sb[:].rearrange("k (t m) -> k t m", t=4),
            rep.ap().rearrange("t k m -> k t m"),
        )
        fcw_sb = const.tile([128, 128], f32, tag="fcw_sb")
        nc.sync.dma_start(fcw_sb[:], fcw.ap())

        # ---- k / v loads: SBUF layout [c(+ones row), (j, h, w)] ----
        # The ones row (for the folded bias) ships inside kx/vx as channel CF.
        k_sb = data.tile([CF + 1, L * HW], f32, tag="k_sb")
        v_sb = data.tile([CF + 1, L * HW], f32, tag="v_sb")
        nc.sync.dma_start(
            k_sb[:].rearrange("c (j p) -> c j p", j=L),
            kx.ap().rearrange("j c p -> c j p"),
        )
        nc.sync.dma_start(
            v_sb[:].rearrange("c (j p) -> c j p", j=L),
            vx.ap().rearrange("j c p -> c j p"),
        )

        def proj(x_sb, wT_sb, out_tag):
            """Per-w-column projection + leaky relu.

            Returns 4 SBUF tiles [128=(w_sub,c), (h, j)], tile t covering w=4t..4t+3.
            """
            outs = []
            xv = x_sb[:].rearrange("c (j h w) -> c j h w", j=L, h=H, w=W)
            for t in range(4):
                pk = ps_proj.tile([128, HJ], f32, tag="pk")
                for ws in range(4):
                    w = 4 * t + ws
                    rhs = xv[:, :, :, w].rearrange("c j h -> c h j")
                    nc.tensor.matmul(
                        pk[ws * 32 : (ws + 1) * 32, :],
                        wT_sb[:],
                        rhs,
                        start=True,
                        stop=True,
                        tile_position=(0, ws * 32),
                    )
                o = data.tile([128, HJ], f32, tag=f"{out_tag}{t}")
                # leaky relu: max(x, 0.1x) = 0.55*x + 0.45*|x|; only one PSUM
                # input is allowed per DVE op, so route |x| through ScalarE.
                ab = tmpp.tile([128, HJ], f32, tag="abs")
                nc.scalar.activation(ab[:], pk[:], Act.Abs, bias=0.0, scale=0.45)
                nc.vector.scalar_tensor_tensor(
                    o[:], pk[:], 0.55, ab[:], op0=Alu.mult, op1=Alu.add
                )
                outs.append(o)
            return outs

        khW = proj(k_sb, wkT_sb, "khW")
        vhW = proj(v_sb, wvT_sb, "vhW")

        # ---- conv stage W: P1[(dh 0..3 @ 32-aligned, w_out), (h, j)], P2[dh=4] ----
        # Engine partition reads must start 32-aligned, so each dh block gets
        # its own 32-partition slot (16 used + 16 pad).
        P1 = ps_p.tile([128, HJ], f32, tag="P1")
        P2 = ps_p.tile([16, HJ], f32, tag="P2")
        for t in range(4):
            nc.tensor.matmul(
                P1[:],
                cw_sb[:, t * 144 : t * 144 + 128],
                khW[t][:],
                start=(t == 0),
                stop=(t == 3),
            )
        for t in range(4):
            nc.tensor.matmul(
                P2[:],
                cw_sb[:, t * 144 + 128 : (t + 1) * 144],
                khW[t][:],
                start=(t == 0),
                stop=(t == 3),
            )

        # ---- conv stage H: shifted add over dh -> Blog [16 w, (h, j)] ----
        Blog = data.tile([W, HJ], f32, tag="Blog")
        nc.scalar.copy(Blog[:], P1[64:80, :])
        Bv = Blog[:].rearrange("p (h j) -> p h j", h=H)
        for dh in (0, 1, 3, 4):
            h_lo = max(0, 2 - dh)
            h_hi = min(16, 18 - dh)
            s_lo = h_lo + dh - 2
            src = P2 if dh == 4 else P1
            p_off = 0 if dh == 4 else dh * 32
            Pv = src[p_off : p_off + 16, :].rearrange("p (h j) -> p h j", h=H)
            nc.vector.tensor_add(
                Bv[:, h_lo:h_hi, :],
                Bv[:, h_lo:h_hi, :],
                Pv[:, s_lo : s_lo + (h_hi - h_lo), :],
            )

        # ---- softmax pieces: ES = [exp(Blog) | sum_j exp] ----
        ES = data.tile([W, HJ + H], f32, tag="ES")
        nc.scalar.activation(ES[:, 0:HJ], Blog[:], Act.Exp)
        nc.vector.tensor_reduce(
            ES[:, HJ : HJ + H],
            ES[:, 0:HJ].rearrange("p (h j) -> p h j", h=H),
            axis=Ax.X,
            op=Alu.add,
        )
        nc.sync.dma_start(es.ap()[:], ES[:])

        # ---- replicate E||S across (w_sub, c) partitions; contract; fc ----
        for t in range(4):
            er = ps_er.tile([128, HJ + H], f32, tag="er")
            nc.tensor.matmul(
                er[:], rep_sb[:, t * 128 : (t + 1) * 128], ES[:], start=True, stop=True
            )
            tmp = tmpp.tile([128, HJ], f32, tag="tmp")
            nc.vector.tensor_mul(tmp[:], vhW[t][:], er[:, 0:HJ])
            res = tmpp.tile([128, H], f32, tag="res")
            nc.vector.tensor_reduce(
                res[:],
                tmp[:].rearrange("p (h j) -> p h j", h=H),
                axis=Ax.X,
                op=Alu.add,
            )
            O = ps_o.tile([128, H], f32, tag="O")
            nc.tensor.matmul(O[:], fcw_sb[:], res[:], start=True, stop=True)
            sr = tmpp.tile([128, H], f32, tag="sr")
            nc.vector.reciprocal(sr[:], er[:, HJ : HJ + H])
            on = tmpp.tile([128, H], f32, tag="on")
            nc.vector.tensor_mul(on[:], O[:], sr[:])
            nc.sync.dma_start(
                outp.ap()[4 * t : 4 * t + 4].rearrange("w o h -> (w o) h"), on[:]
            )

    nc.compile()
    return nc


def _make_tables(wk, bk, wv, bv, w_attn, w_fc, n):
    """Host-side per-core constant tables for head n."""
    f = np.float32
    wkT = np.concatenate(
        [wk[n * DM : (n + 1) * DM, :].T, bk[None, n * DM : (n + 1) * DM]], axis=0
    ).astype(f)
    wvT = np.concatenate(
        [wv[n * DM : (n + 1) * DM, :].T, bv[None, n * DM : (n + 1) * DM]], axis=0
    ).astype(f)

    wa = w_attn[0]  # (32, 5, 5)
    # cw[t] columns: [dh0 @0, dh1 @32, dh2 @64, dh3 @96 (16 used + 16 pad each),
    #                 dh4 @128..144]
    cw = np.zeros((4, 128, 144), f)
    for t in range(4):
        for ws in range(4):
            w = 4 * t + ws
            for wo in range(W):
                dw = w - wo + 2
                if 0 <= dw < 5:
                    for dh in range(5):
                        col = 128 + wo if dh == 4 else dh * 32 + wo
                        cw[t, ws * 32 : (ws + 1) * 32, col] = wa[:, dh, dw]

    rep = np.zeros((4, 16, 128), f)
    for t in range(4):
        for ws in range(4):
            rep[t, 4 * t + ws, ws * 32 : (ws + 1) * 32] = 1.0

    fcw = np.zeros((128, 128), f)
    blk = w_fc[:, n * DM : (n + 1) * DM]  # (o, c)
    for ws in range(4):
        fcw[ws * 32 : (ws + 1) * 32, ws * 32 : (ws + 1) * 32] = blk.T
    return wkT, wvT, cw, rep, fcw


def _get_runner():
    global _RUNNER
    if _RUNNER is None:
        _RUNNER = _build_nc()
    return _RUNNER


def run_on_device(in_maps, trace=False):
    from concourse.bass_utils import run_bass_kernel_spmd

    nc = _get_runner()
    return run_bass_kernel_spmd(nc, in_maps, list(range(N_CORES)), trace=trace)


def make_in_maps(k, v, wk, bk, wv, bv, w_attn, w_fc):
    k = np.asarray(k, np.float32)
    v = np.asarray(v, np.float32)
    in_maps = []
    for core in range(N_CORES):
        b, n = divmod(core, NH)
        wkT, wvT, cw, rep, fcw = _make_tables(
            np.asarray(wk, np.float32),
            np.asarray(bk, np.float32),
            np.asarray(wv, np.float32),
            np.asarray(bv, np.float32),
            np.asarray(w_attn, np.float32),
            np.asarray(w_fc, np.float32),
            n,
        )
        ones = np.ones((L, 1, HW), np.float32)
        in_maps.append(
            {
                "kx": np.ascontiguousarray(
                    np.concatenate([k[b].reshape(L, CF, HW), ones], axis=1)
                ),
                "vx": np.ascontiguousarray(
                    np.concatenate([v[b].reshape(L, CF, HW), ones], axis=1)
                ),
                "wkT": wkT,
                "wvT": wvT,
                "cw": cw,
                "rep": rep,
                "fcw": fcw,
            }
        )
    return in_maps


def assemble_outputs(results):
    out1 = np.zeros((B, DM, H, W), np.float32)
    attn1 = np.zeros((B, NH, L, H, W), np.float32)
    for core in range(N_CORES):
        b, n = divmod(core, NH)
        op = results[core]["outp"]  # (w, o, h)
        out1[b] += op.transpose(1, 2, 0)
        es_r = results[core]["es"]  # (w, 384+16)
        E = es_r[:, :HJ].reshape(W, H, L)  # (w, h, j)
        S = es_r[:, HJ:]  # (w, h)
        attn1[b, n] = (E / S[:, :, None]).transpose(2, 1, 0)  # (j, h, w)

    out = np.ascontiguousarray(
        np.broadcast_to(out1[:, None], (B, L, DM, H, W)), dtype=np.float32
    )
    attn = np.ascontiguousarray(
        np.broadcast_to(attn1[:, :, None, :, None], (B, NH, L, L, 1, H, W)),
        dtype=np.float32,
    )
    return out, attn


def kernel(q, k, v, wq, bq, wk, bk, wv, bv, w_attn, b_attn, w_fc, _trace=False):
    del q, wq, bq, b_attn  # output is provably independent of these
    in_maps = make_in_maps(k, v, wk, bk, wv, bv, w_attn, w_fc)
    br = run_on_device(in_maps, trace=_trace)
    out, attn = assemble_outputs(br.results)
    if _trace:
        return (out, attn), br
    return out, attn


# revision 27
# speedup vs baseline: 1.4479x; 1.4479x over previous
"""Trainium2 Bass kernel for nn_MultiHeadConvAttention.

Math collapse used here (verified against the reference):
  - The 5x5 attention conv is linear, so conv(qh_i + kh_j) = conv(qh_i) + conv(kh_j).
  - Softmax over keys j is invariant to the query-side term conv(qh_i) (constant in j)
    and to b_attn, so attn[b,n,i,j,h,w] = softmax_j(conv(kh)[b,n,j,h,w]) — independent
    of i and of q entirely. Likewise the weighted V-sum and the final fc are
    independent of the query index, so out[b,q,...] is constant over q.

Sharding: core = b*4 + n handles one (batch, head) pair. Device work per core:
  kh/vh projections with bias folded in as an extra contraction row (per-w-column
  matmuls into a (w_sub, c)-partition layout), leaky-relu on DVE, the 5x5 conv as a
  W-Toeplitz matmul over (w_in, c) + a small shifted-add over dh, exp + key-sum,
  a replication matmul, the attn*V contraction on DVE, and the final fc as a
  block-diagonal matmul. Unnormalized exp E and key-sums S are returned;
  normalization/broadcast of the attn output happens on the host.

SBUF free layout convention is (h, j): innermost j so key-reductions are contiguous.
"""

import sys

for _p in ("/opt/trn_rl_repo", "/opt/pypackages"):
    if _p not in sys.path:
        sys.path.insert(0, _p)

import numpy as np

B, L, CF, NH, DM, H, W = 2, 24, 32, 4, 32, 16, 16
HW = H * W
HJ = H * L  # 384, free size of (h, j)
N_CORES = 8

_RUNNER = None


def _build_nc():
    import concourse.bacc as bacc
    import concourse.mybir as mybir
    import concourse.tile as tile
    from contextlib import ExitStack

    f32 = mybir.dt.float32
    f32r = mybir.dt.float32r
    bf16 = mybir.dt.bfloat16
    nc = bacc.Bacc("TRN2", target_bir_lowering=False, debug=False)

    kx = nc.dram_tensor("kx", [L, CF + 1, HW], bf16, kind="ExternalInput")
    vx = nc.dram_tensor("vx", [L, CF + 1, HW], bf16, kind="ExternalInput")
    wkT = nc.dram_tensor("wkT", [CF + 1, DM], bf16, kind="ExternalInput")
    wvT = nc.dram_tensor("wvT", [CF + 1, DM], bf16, kind="ExternalInput")
    cw = nc.dram_tensor("cw", [4, 128, 144], f32r, kind="ExternalInput")
    rep = nc.dram_tensor("rep", [4, 16, 128], f32r, kind="ExternalInput")
    fcw = nc.dram_tensor("fcw", [128, 128], f32, kind="ExternalInput")
    outp = nc.dram_tensor("outp", [W, DM, H], f32, kind="ExternalOutput")
    es = nc.dram_tensor("es", [W, HJ + H], f32r, kind="ExternalOutput")

    Act = mybir.ActivationFunctionType
    Alu = mybir.AluOpType
    Ax = mybir.AxisListType


    with tile.TileContext(nc) as tc, ExitStack() as ctx:
        const = ctx.enter_context(tc.tile_pool(name="const", bufs=1))
        data = ctx.enter_context(tc.tile_pool(name="data", bufs=1))
        tmpp = ctx.enter_context(tc.tile_pool(name="tmpp", bufs=2))
        ps_proj = ctx.enter_context(tc.tile_pool(name="ps_proj", bufs=2, space="PSUM"))
        ps_big = ctx.enter_context(tc.tile_pool(name="ps_big", bufs=3, space="PSUM"))
        ps_o = ctx.enter_context(tc.tile_pool(name="ps_o", bufs=1, space="PSUM"))

        # ---- loads, in rough order of first use: proj weights, k, tables, v
        wkT_sb = const.tile([CF + 1, DM], bf16, tag="wkT_sb")
        nc.sync.dma_start(wkT_sb[:], wkT.ap())
        wvT_sb = const.tile([CF + 1, DM], bf16, tag="wvT_sb")
        nc.sync.dma_start(wvT_sb[:], wvT.ap())

        # k / v SBUF layout [c(+ones row), (j, h, w)]; the ones row (for the
        # folded bias) ships inside kx/vx as channel CF.
        k_sb = data.tile([CF + 1, L * HW], bf16, tag="k_sb")
        v_sb = data.tile([CF + 1, L * HW], bf16, tag="v_sb")
        nc.sync.dma_start(
            k_sb[:].rearrange("c (j p) -> c j p", j=L),
            kx.ap().rearrange("j c p -> c j p"),
        )

        cw_sb = const.tile([128, 4 * 144], f32r, tag="cw_sb")
        nc.sync.dma_start(
            cw_sb[:].rearrange("k (t m) -> k t m", t=4),
            cw.ap().rearrange("t k m -> k t m"),
        )
        rep_sb = const.tile([16, 4 * 128], f32r, tag="rep_sb")
        nc.sync.dma_start(
            rep_sb[:].rearrange("k (t m) -> k t m", t=4),
            rep.ap().rearrange("t k m -> k t m"),
        )
        fcw_sb = const.tile([128, 128], f32, tag="fcw_sb")
        nc.sync.dma_start(fcw_sb[:], fcw.ap())

        nc.sync.dma_start(
            v_sb[:].rearrange("c (j p) -> c j p", j=L),
            vx.ap().rearrange("j c p -> c j p"),
        )

        def proj(x_sb, wT_sb, out_tag, out_dt):
            """Per-w-column projection + leaky relu.

            Returns 4 SBUF tiles [128=(w_sub,c), (h, j)], tile t covering w=4t..4t+3.
            """
            outs = []
            xv = x_sb[:].rearrange("c (j h w) -> c j h w", j=L, h=H, w=W)
            for t in range(4):
                pk = ps_proj.tile([128, HJ], f32, tag="pk")
                for ws in range(4):
                    w = 4 * t + ws
                    rhs = xv[:, :, :, w].rearrange("c j h -> c h j")
                    nc.tensor.matmul(
                        pk[ws * 32 : (ws + 1) * 32, :],
                        wT_sb[:],
                        rhs,
                        start=True,
                        stop=True,
                        tile_position=(0, ws * 32),
                    )
                o = data.tile([128, HJ], out_dt, tag=f"{out_tag}{t}")
                # leaky relu: max(x, 0.1x) = 0.55*x + 0.45*|x|; only one PSUM
                # input is allowed per DVE op, so route |x| through ScalarE.
                ab = tmpp.tile([128, HJ], f32, tag="abs")
                nc.scalar.activation(ab[:], pk[:], Act.Abs, bias=0.0, scale=0.45)
                nc.vector.scalar_tensor_tensor(
                    o[:], pk[:], 0.55, ab[:], op0=Alu.mult, op1=Alu.add
                )
                outs.append(o)
            return outs

        khW = proj(k_sb, wkT_sb, "khW", f32r)
        vhW = proj(v_sb, wvT_sb, "vhW", f32)

        # ---- conv stage W: P1[(dh 0..3 @ 32-aligned, w_out), (h, j)], P2[dh=4] ----
        # Engine partition reads must start 32-aligned, so each dh block gets
        # its own 32-partition slot (16 used + 16 pad).
        P1 = ps_big.tile([128, HJ], f32, tag="big")
        P2 = ps_big.tile([128, HJ], f32, tag="big")
        for t in range(4):
            nc.tensor.matmul(
                P1[:],
                cw_sb[:, t * 144 : t * 144 + 128],
                khW[t][:],
                start=(t == 0),
                stop=(t == 3),
            )
        for t in range(4):
            nc.tensor.matmul(
                P2[:16, :],
                cw_sb[:, t * 144 + 128 : (t + 1) * 144],
                khW[t][:],
                start=(t == 0),
                stop=(t == 3),
            )

        # ---- conv stage H: shifted add over dh -> Blog [16 w, (h, j)] ----
        Blog = data.tile([W, HJ], f32, tag="Blog")
        nc.scalar.copy(Blog[:], P1[64:80, :])
        Bv = Blog[:].rearrange("p (h j) -> p h j", h=H)
        for dh in (0, 1, 3, 4):
            h_lo = max(0, 2 - dh)
            h_hi = min(16, 18 - dh)
            s_lo = h_lo + dh - 2
            src = P2[:16, :] if dh == 4 else P1[:]
            p_off = 0 if dh == 4 else dh * 32
            Pv = src[p_off : p_off + 16, :].rearrange("p (h j) -> p h j", h=H)
            nc.vector.tensor_add(
                Bv[:, h_lo:h_hi, :],
                Bv[:, h_lo:h_hi, :],
                Pv[:, s_lo : s_lo + (h_hi - h_lo), :],
            )

        # ---- softmax pieces: ES = [exp(Blog) | sum_j exp] ----
        ES = data.tile([W, HJ + H], f32r, tag="ES")
        nc.scalar.activation(ES[:, 0:HJ], Blog[:], Act.Exp)
        with nc.allow_low_precision(reason="f32r key-sum; ~1e-5 rel is fine"):
            nc.vector.tensor_reduce(
                ES[:, HJ : HJ + H],
                ES[:, 0:HJ].rearrange("p (h j) -> p h j", h=H),
                axis=Ax.X,
                op=Alu.add,
            )
        nc.sync.dma_start(es.ap()[:], ES[:])

        # ---- replicate E||S across (w_sub, c) partitions; contract; fc ----
        for t in range(4):
            er = ps_big.tile([128, HJ + H], f32, tag="big")
            nc.tensor.matmul(
                er[:],
                rep_sb[:, t * 128 : (t + 1) * 128],
                ES[:],
                start=True,
                stop=True,
            )
            tmp = tmpp.tile([128, HJ], f32, tag="tmp")
            nc.vector.tensor_mul(tmp[:], vhW[t][:], er[:, 0:HJ])
            res = tmpp.tile([128, H], f32, tag="res")
            nc.vector.tensor_reduce(
                res[:],
                tmp[:].rearrange("p (h j) -> p h j", h=H),
                axis=Ax.X,
                op=Alu.add,
            )
            O = ps_o.tile([128, H], f32, tag="O")
            nc.tensor.matmul(O[:], fcw_sb[:], res[:], start=True, stop=True)
            sr = tmpp.tile([128, H], f32, tag="sr")
            nc.vector.reciprocal(sr[:], er[:, HJ : HJ + H])
            on = tmpp.tile([128, H], f32, tag="on")
            nc.vector.tensor_mul(on[:], O[:], sr[:])
            nc.sync.dma_start(
                outp.ap()[4 * t : 4 * t + 4].rearrange("w o h -> (w o) h"), on[:]
            )

    nc.compile()
    return nc


def _make_tables(wk, bk, wv, bv, w_attn, w_fc, n):
    """Host-side per-core constant tables for head n."""
    import ml_dtypes

    f = np.float32
    wkT = np.concatenate(
        [wk[n * DM : (n + 1) * DM, :].T, bk[None, n * DM : (n + 1) * DM]], axis=0
    ).astype(ml_dtypes.bfloat16)
    wvT = np.concatenate(
        [wv[n * DM : (n + 1) * DM, :].T, bv[None, n * DM : (n + 1) * DM]], axis=0
    ).astype(ml_dtypes.bfloat16)

    wa = w_attn[0]  # (32, 5, 5)
    # cw[t] columns: [dh0 @0, dh1 @32, dh2 @64, dh3 @96 (16 used + 16 pad each),
    #                 dh4 @128..144]
    cw = np.zeros((4, 128, 144), f)
    for t in range(4):
        for ws in range(4):
            w = 4 * t + ws
            for wo in range(W):
                dw = w - wo + 2
                if 0 <= dw < 5:
                    for dh in range(5):
                        col = 128 + wo if dh == 4 else dh * 32 + wo
                        cw[t, ws * 32 : (ws + 1) * 32, col] = wa[:, dh, dw]

    rep = np.zeros((4, 16, 128), f)
    for t in range(4):
        for ws in range(4):
            rep[t, 4 * t + ws, ws * 32 : (ws + 1) * 32] = 1.0

    fcw = np.zeros((128, 128), f)
    blk = w_fc[:, n * DM : (n + 1) * DM]  # (o, c)
    for ws in range(4):
        fcw[ws * 32 : (ws + 1) * 32, ws * 32 : (ws + 1) * 32] = blk.T
    return wkT, wvT, cw, rep, fcw


def _get_runner():
    global _RUNNER
    if _RUNNER is None:
        _RUNNER = _build_nc()
    return _RUNNER


def run_on_device(in_maps, trace=False):
    from concourse.bass_utils import run_bass_kernel_spmd

    nc = _get_runner()
    return run_bass_kernel_spmd(nc, in_maps, list(range(N_CORES)), trace=trace)


def make_in_maps(k, v, wk, bk, wv, bv, w_attn, w_fc):
    k = np.asarray(k, np.float32)
    v = np.asarray(v, np.float32)
    in_maps = []
    for core in range(N_CORES):
        b, n = divmod(core, NH)
        wkT, wvT, cw, rep, fcw = _make_tables(
            np.asarray(wk, np.float32),
            np.asarray(bk, np.float32),
            np.asarray(wv, np.float32),
            np.asarray(bv, np.float32),
            np.asarray(w_attn, np.float32),
            np.asarray(w_fc, np.float32),
            n,
        )
        import ml_dtypes

        ones = np.ones((L, 1, HW), np.float32)
        in_maps.append(
            {
                "kx": np.ascontiguousarray(
                    np.concatenate([k[b].reshape(L, CF, HW), ones], axis=1)
                ).astype(ml_dtypes.bfloat16),
                "vx": np.ascontiguousarray(
                    np.concatenate([v[b].reshape(L, CF, HW), ones], axis=1)
                ).astype(ml_dtypes.bfloat16),
                "wkT": wkT,
                "wvT": wvT,
                "cw": cw,
                "rep": rep,
                "fcw": fcw,
            }
        )
    return in_maps


def assemble_outputs(results):
    out1 = np.zeros((B, DM, H, W), np.float32)
    attn1 = np.zeros((B, NH, L, H, W), np.float32)
    for core in range(N_CORES):
        b, n = divmod(core, NH)
        op = results[core]["outp"]  # (w, o, h)
        out1[b] += op.transpose(1, 2, 0)
        es_r = results[core]["es"]  # (w, 384+16)
        E = es_r[:, :HJ].reshape(W, H, L)  # (w, h, j)
        S = es_r[:, HJ:]  # (w, h)
        attn1[b, n] = (E / S[:, :, None]).transpose(2, 1, 0)  # (j, h, w)

    out = np.ascontiguousarray(
        np.broadcast_to(out1[:, None], (B, L, DM, H, W)), dtype=np.float32
    )
    attn = np.ascontiguousarray(
        np.broadcast_to(attn1[:, :, None, :, None], (B, NH, L, L, 1, H, W)),
        dtype=np.float32,
    )
    return out, attn


def kernel(q, k, v, wq, bq, wk, bk, wv, bv, w_attn, b_attn, w_fc, _trace=False):
    del q, wq, bq, b_attn  # output is provably independent of these
    in_maps = make_in_maps(k, v, wk, bk, wv, bv, w_attn, w_fc)
    br = run_on_device(in_maps, trace=_trace)
    out, attn = assemble_outputs(br.results)
    if _trace:
        return (out, attn), br
    return out, attn


# revision 29
# speedup vs baseline: 1.6387x; 1.1318x over previous
"""Trainium2 Bass kernel for nn_MultiHeadConvAttention.

Math collapse used here (verified against the reference):
  - The 5x5 attention conv is linear, so conv(qh_i + kh_j) = conv(qh_i) + conv(kh_j).
  - Softmax over keys j is invariant to the query-side term conv(qh_i) (constant in j)
    and to b_attn, so attn[b,n,i,j,h,w] = softmax_j(conv(kh)[b,n,j,h,w]) — independent
    of i and of q entirely. Likewise the weighted V-sum and the final fc are
    independent of the query index, so out[b,q,...] is constant over q.

Sharding: core = b*4 + n handles one (batch, head) pair. Device work per core:
  kh/vh projections with bias folded in as an extra contraction row (per-w-column
  matmuls into a (w_sub, c)-partition layout), leaky-relu on DVE, the 5x5 conv as a
  W-Toeplitz matmul over (w_in, c) + a small shifted-add over dh, exp + key-sum,
  a replication matmul, the attn*V contraction on DVE, and the final fc as a
  block-diagonal matmul. Unnormalized exp E and key-sums S are returned;
  normalization/broadcast of the attn output happens on the host.

SBUF free layout convention is (h, j): innermost j so key-reductions are contiguous.
"""

import sys

for _p in ("/opt/trn_rl_repo", "/opt/pypackages"):
    if _p not in sys.path:
        sys.path.insert(0, _p)

import numpy as np

B, L, CF, NH, DM, H, W = 2, 24, 32, 4, 32, 16, 16
HW = H * W
HJ = H * L  # 384, free size of (h, j)
N_CORES = 8

_RUNNER = None


def _build_nc():
    import concourse.bacc as bacc
    import concourse.mybir as mybir
    import concourse.tile as tile
    from contextlib import ExitStack

    f32 = mybir.dt.float32
    f32r = mybir.dt.float32r
    bf16 = mybir.dt.bfloat16
    nc = bacc.Bacc("TRN2", target_bir_lowering=False, debug=False)

    # kx rows (c, + ones row): [k in (w, h, j) order | wkT col | wvT col]
    kx = nc.dram_tensor("kx", [CF + 1, L * HW + 2 * DM], bf16, kind="ExternalInput")
    vx = nc.dram_tensor("vx", [CF + 1, L * HW], bf16, kind="ExternalInput")
    # blob rows: [cw (4 tiles x 144) | fcw (128, f32 bytes)]
    blob = nc.dram_tensor("blob", [128, 4 * 144 + 128], f32r, kind="ExternalInput")
    rep = nc.dram_tensor("rep", [16, 4 * 128], f32r, kind="ExternalInput")
    outp = nc.dram_tensor("outp", [W, DM, H], f32, kind="ExternalOutput")
    es = nc.dram_tensor("es", [W, HJ + H], f32r, kind="ExternalOutput")

    Act = mybir.ActivationFunctionType
    Alu = mybir.AluOpType
    Ax = mybir.AxisListType


    with tile.TileContext(nc) as tc, ExitStack() as ctx:
        const = ctx.enter_context(tc.tile_pool(name="const", bufs=1))
        data = ctx.enter_context(tc.tile_pool(name="data", bufs=1))
        tmpp = ctx.enter_context(tc.tile_pool(name="tmpp", bufs=2))
        ps_proj = ctx.enter_context(tc.tile_pool(name="ps_proj", bufs=3, space="PSUM"))
        ps_big = ctx.enter_context(tc.tile_pool(name="ps_big", bufs=4, space="PSUM"))
        ps_o = ctx.enter_context(tc.tile_pool(name="ps_o", bufs=1, space="PSUM"))

        # ---- loads: 4 fully contiguous DMAs ----
        # k/v ship pre-transposed to [c(+ones), (w, h, j)] so both the DMA and
        # the per-w proj rhs slices are contiguous; wkT/wvT ride in kx's rows.
        k_sb = data.tile([CF + 1, L * HW + 2 * DM], bf16, tag="k_sb")
        v_sb = data.tile([CF + 1, L * HW], bf16, tag="v_sb")
        nc.sync.dma_start(k_sb[:], kx.ap())
        blob_sb = const.tile([128, 4 * 144 + 128], f32r, tag="blob_sb")
        nc.sync.dma_start(blob_sb[:], blob.ap())
        rep_sb = const.tile([16, 4 * 128], f32r, tag="rep_sb")
        nc.sync.dma_start(rep_sb[:], rep.ap())
        nc.sync.dma_start(v_sb[:], vx.ap())

        wkT_sb = k_sb[:, L * HW : L * HW + DM]
        wvT_sb = k_sb[:, L * HW + DM : L * HW + 2 * DM]
        cw_sb = blob_sb[:, 0 : 4 * 144]
        fcw_sb = blob_sb[:, 4 * 144 : 4 * 144 + 128].bitcast(f32)

        def proj(x_sb, wT_sb, out_tag, out_dt):
            """Per-w-column projection + leaky relu.

            Returns 4 SBUF tiles [128=(w_sub,c), (h, j)], tile t covering w=4t..4t+3.
            """
            outs = []
            for t in range(4):
                pk = ps_proj.tile([128, HJ], f32, tag="pk")
                for ws in range(4):
                    w = 4 * t + ws
                    rhs = x_sb[:, : L * HW].rearrange("c (w hj) -> c w hj", w=W)[
                        :, w, :
                    ]
                    nc.tensor.matmul(
                        pk[ws * 32 : (ws + 1) * 32, :],
                        wT_sb,
                        rhs,
                        start=True,
                        stop=True,
                        tile_position=(0, ws * 32),
                    )
                o = data.tile([128, HJ], out_dt, tag=f"{out_tag}{t}")
                # leaky relu: max(x, 0.1x) = 0.55*x + 0.45*|x|; only one PSUM
                # input is allowed per DVE op, so route |x| through ScalarE.
                ab = tmpp.tile([128, HJ], f32, tag="abs")
                nc.scalar.activation(ab[:], pk[:], Act.Abs, bias=0.0, scale=0.45)
                nc.vector.scalar_tensor_tensor(
                    o[:], pk[:], 0.55, ab[:], op0=Alu.mult, op1=Alu.add
                )
                outs.append(o)
            return outs

        khW = proj(k_sb, wkT_sb, "khW", f32r)
        vhW = proj(v_sb, wvT_sb, "vhW", f32)

        # ---- conv stage W: P1[(dh 0..3 @ 32-aligned, w_out), (h, j)], P2[dh=4] ----
        # Engine partition reads must start 32-aligned, so each dh block gets
        # its own 32-partition slot (16 used + 16 pad).
        P1 = ps_big.tile([128, HJ], f32, tag="big")
        P2 = ps_big.tile([128, HJ], f32, tag="big")
        for t in range(4):
            nc.tensor.matmul(
                P1[:],
                cw_sb[:, t * 144 : t * 144 + 128],
                khW[t][:],
                start=(t == 0),
                stop=(t == 3),
            )
        for t in range(4):
            nc.tensor.matmul(
                P2[:16, :],
                cw_sb[:, t * 144 + 128 : (t + 1) * 144],
                khW[t][:],
                start=(t == 0),
                stop=(t == 3),
            )

        # ---- conv stage H: shifted add over dh -> Blog [16 w, (h, j)] ----
        Blog = data.tile([W, HJ], f32, tag="Blog")
        nc.scalar.copy(Blog[:], P1[64:80, :])
        Bv = Blog[:].rearrange("p (h j) -> p h j", h=H)
        for dh in (0, 1, 3, 4):
            h_lo = max(0, 2 - dh)
            h_hi = min(16, 18 - dh)
            s_lo = h_lo + dh - 2
            src = P2[:16, :] if dh == 4 else P1[:]
            p_off = 0 if dh == 4 else dh * 32
            Pv = src[p_off : p_off + 16, :].rearrange("p (h j) -> p h j", h=H)
            nc.vector.tensor_add(
                Bv[:, h_lo:h_hi, :],
                Bv[:, h_lo:h_hi, :],
                Pv[:, s_lo : s_lo + (h_hi - h_lo), :],
            )

        # ---- softmax pieces: ES = [exp(Blog) | sum_j exp] ----
        ES = data.tile([W, HJ + H], f32r, tag="ES")
        nc.scalar.activation(ES[:, 0:HJ], Blog[:], Act.Exp)
        with nc.allow_low_precision(reason="f32r key-sum; ~1e-5 rel is fine"):
            nc.vector.tensor_reduce(
                ES[:, HJ : HJ + H],
                ES[:, 0:HJ].rearrange("p (h j) -> p h j", h=H),
                axis=Ax.X,
                op=Alu.add,
            )
        nc.sync.dma_start(es.ap()[:], ES[:])

        # ---- replicate E||S across (w_sub, c) partitions; contract; fc ----
        for t in range(4):
            er = ps_big.tile([128, HJ + H], f32, tag="big")
            nc.tensor.matmul(
                er[:],
                rep_sb[:, t * 128 : (t + 1) * 128],
                ES[:],
                start=True,
                stop=True,
            )
            tmp = tmpp.tile([128, HJ], f32, tag="tmp")
            nc.vector.tensor_mul(tmp[:], vhW[t][:], er[:, 0:HJ])
            res = tmpp.tile([128, H], f32, tag="res")
            nc.vector.tensor_reduce(
                res[:],
                tmp[:].rearrange("p (h j) -> p h j", h=H),
                axis=Ax.X,
                op=Alu.add,
            )
            O = ps_o.tile([128, H], f32, tag="O")
            nc.tensor.matmul(O[:], fcw_sb, res[:], start=True, stop=True)
            sr = tmpp.tile([128, H], f32, tag="sr")
            nc.vector.reciprocal(sr[:], er[:, HJ : HJ + H])
            on = tmpp.tile([128, H], f32, tag="on")
            nc.vector.tensor_mul(on[:], O[:], sr[:])
            nc.sync.dma_start(
                outp.ap()[4 * t : 4 * t + 4].rearrange("w o h -> (w o) h"), on[:]
            )

    nc.compile()
    return nc


def _make_tables(wk, bk, wv, bv, w_attn, w_fc, n):
    """Host-side per-core constant tables for head n.

    Returns (wkT, wvT) bf16 [33, 32] each, blob [128, 704] f32 (cw | fcw),
    rep [16, 512] f32.
    """
    import ml_dtypes

    f = np.float32
    wkT = np.concatenate(
        [wk[n * DM : (n + 1) * DM, :].T, bk[None, n * DM : (n + 1) * DM]], axis=0
    ).astype(ml_dtypes.bfloat16)
    wvT = np.concatenate(
        [wv[n * DM : (n + 1) * DM, :].T, bv[None, n * DM : (n + 1) * DM]], axis=0
    ).astype(ml_dtypes.bfloat16)

    wa = w_attn[0]  # (32, 5, 5)
    # cw[t] columns: [dh0 @0, dh1 @32, dh2 @64, dh3 @96 (16 used + 16 pad each),
    #                 dh4 @128..144]
    cw = np.zeros((4, 128, 144), f)
    for t in range(4):
        for ws in range(4):
            w = 4 * t + ws
            for wo in range(W):
                dw = w - wo + 2
                if 0 <= dw < 5:
                    for dh in range(5):
                        col = 128 + wo if dh == 4 else dh * 32 + wo
                        cw[t, ws * 32 : (ws + 1) * 32, col] = wa[:, dh, dw]

    rep = np.zeros((4, 16, 128), f)
    for t in range(4):
        for ws in range(4):
            rep[t, 4 * t + ws, ws * 32 : (ws + 1) * 32] = 1.0

    fcw = np.zeros((128, 128), f)
    blk = w_fc[:, n * DM : (n + 1) * DM]  # (o, c)
    for ws in range(4):
        fcw[ws * 32 : (ws + 1) * 32, ws * 32 : (ws + 1) * 32] = blk.T

    blob = np.concatenate([cw.transpose(1, 0, 2).reshape(128, 4 * 144), fcw], axis=1)
    rep2 = np.ascontiguousarray(rep.transpose(1, 0, 2).reshape(16, 4 * 128))
    return wkT, wvT, blob, rep2


def _pack_kv(x_b, wT1, wT2):
    """(L, CF, H, W) + ones channel -> [33, (w, h, j)] bf16, optionally with
    weight columns appended."""
    import ml_dtypes

    arr = np.concatenate(
        [x_b.reshape(L, CF, H, W), np.ones((L, 1, H, W), np.float32)], axis=1
    )
    # (L, 33, H, W) -> (33, W, H, L)
    arr = arr.transpose(1, 3, 2, 0).reshape(CF + 1, L * HW)
    cols = [arr.astype(ml_dtypes.bfloat16)]
    if wT1 is not None:
        cols += [wT1, wT2]
    return np.ascontiguousarray(np.concatenate(cols, axis=1))


def _get_runner():
    global _RUNNER
    if _RUNNER is None:
        _RUNNER = _build_nc()
    return _RUNNER


def run_on_device(in_maps, trace=False):
    from concourse.bass_utils import run_bass_kernel_spmd

    nc = _get_runner()
    return run_bass_kernel_spmd(nc, in_maps, list(range(N_CORES)), trace=trace)


def make_in_maps(k, v, wk, bk, wv, bv, w_attn, w_fc):
    k = np.asarray(k, np.float32)
    v = np.asarray(v, np.float32)
    in_maps = []
    for core in range(N_CORES):
        b, n = divmod(core, NH)
        wkT, wvT, blob, rep2 = _make_tables(
            np.asarray(wk, np.float32),
            np.asarray(bk, np.float32),
            np.asarray(wv, np.float32),
            np.asarray(bv, np.float32),
            np.asarray(w_attn, np.float32),
            np.asarray(w_fc, np.float32),
            n,
        )
        in_maps.append(
            {
                "kx": _pack_kv(k[b], wkT, wvT),
                "vx": _pack_kv(v[b], None, None),
                "blob": blob,
                "rep": rep2,
            }
        )
    return in_maps


def assemble_outputs(results):
    out1 = np.zeros((B, DM, H, W), np.float32)
    attn1 = np.zeros((B, NH, L, H, W), np.float32)
    for core in range(N_CORES):
        b, n = divmod(core, NH)
        op = results[core]["outp"]  # (w, o, h)
        out1[b] += op.transpose(1, 2, 0)
        es_r = results[core]["es"]  # (w, 384+16)
        E = es_r[:, :HJ].reshape(W, H, L)  # (w, h, j)
        S = es_r[:, HJ:]  # (w, h)
        attn1[b, n] = (E / S[:, :, None]).transpose(2, 1, 0)  # (j, h, w)

    out = np.ascontiguousarray(
        np.broadcast_to(out1[:, None], (B, L, DM, H, W)), dtype=np.float32
    )
    attn = np.ascontiguousarray(
        np.broadcast_to(attn1[:, :, None, :, None], (B, NH, L, L, 1, H, W)),
        dtype=np.float32,
    )
    return out, attn


def kernel(q, k, v, wq, bq, wk, bk, wv, bv, w_attn, b_attn, w_fc, _trace=False):
    del q, wq, bq, b_attn  # output is provably independent of these
    in_maps = make_in_maps(k, v, wk, bk, wv, bv, w_attn, w_fc)
    br = run_on_device(in_maps, trace=_trace)
    out, attn = assemble_outputs(br.results)
    if _trace:
        return (out, attn), br
    return out, attn
